# revision 1
# baseline (speedup 1.0000x reference)
"""CapsuleLayer dynamic-routing kernel for Trainium2 (8 NeuronCores).

Problem: x [256,1152,8] f32, route_weights [10,1152,8,16] f32 ->
out [10,256,1,16] f32 (3 routing iterations, softmax over the 1152
route nodes).

Algebra: logits accumulate additively and each delta is priors .
outputs_t, so logits_t = priors . u_t with u_1 = O_0, u_2 = O_0 + O_1.
Priors are never materialized; each iteration computes V = W_c @ u
(PE, f32r), l = sum_i x * V (DVE mult + strided reduce), e = exp(l)
with fused per-partition sum d (ACT), y^T = x^T * e^T (GPSIMD,
broadcast over i), s^T = sum_k W[k,o] y^T[k,b] (PE, 72 accumulated
matmuls), then squash / u update with tiny per-partition ops.

Sharding: 20 units of (capsule c, batch-half of 128).  Each core gets a
batch-half and 3 c-slots (cores with only 2 real units get a dummy
all-ones weight whose output is discarded).  No cross-core comms.
x^T and W^T layouts are pre-transposed on the host.
"""

import os
import sys

for _p in ("/opt/trn_rl_repo",):
    if _p not in sys.path:
        sys.path.insert(0, _p)

import numpy as np
from contextlib import ExitStack

import concourse.bass as bass
import concourse.tile as tile
from concourse import mybir
from concourse._compat import with_exitstack
from concourse.masks import make_identity

F32 = mybir.dt.float32
F32R = mybir.dt.float32r
AF = mybir.ActivationFunctionType
OP = mybir.AluOpType
AX = mybir.AxisListType

C, B, R, I, O = 10, 256, 1152, 8, 16
K = R * I            # 9216
RB = R // 128        # 9 r-blocks
KT = K // 128        # 72 k-tiles
BH = 128             # batch-half per core
NSLOT = 3            # c-slots per core
NCORES = 8

USE_F32R_V = int(os.environ.get("CAPS_F32R_V", "0"))
USE_F32R_S = int(os.environ.get("CAPS_F32R_S", "0"))
USE_F32R_S2 = int(os.environ.get("CAPS_F32R_S2", "0"))

# core k -> (batch_half, [c0, c1, c2]) ; -1 = dummy slot
CSETS = [[0, 4, 8], [1, 5, 9], [2, 6, -1], [3, 7, -1]]


def core_assignment(k):
    return k // 4, CSETS[k % 4]


@with_exitstack
def _caps_kernel(ctx: ExitStack, tc: tile.TileContext, out_ap, xh, xT_in,
                 w_aps, wT_aps):
    nc = tc.nc

    singles = ctx.enter_context(tc.tile_pool(name="singles", bufs=1))
    tw_pool = ctx.enter_context(tc.tile_pool(name="twave", bufs=2))
    y_pool = ctx.enter_context(tc.tile_pool(name="yhalf", bufs=3))
    le_pool = ctx.enter_context(tc.tile_pool(name="le", bufs=3))
    et_pool = ctx.enter_context(tc.tile_pool(name="et", bufs=2))
    small = ctx.enter_context(tc.tile_pool(name="small", bufs=3))
    psv = ctx.enter_context(tc.tile_pool(name="psv", bufs=2, space="PSUM"))
    pst = ctx.enter_context(tc.tile_pool(name="pst", bufs=2, space="PSUM"))
    pss = ctx.enter_context(tc.tile_pool(name="pss", bufs=2, space="PSUM"))

    def mmdt(ap, fast):
        if fast:
            return ap if ap.dtype == F32R else ap.bitcast(F32R)
        return ap.bitcast(F32) if ap.dtype == F32R else ap

    ident = singles.tile([128, 128], F32)
    make_identity(nc, ident)

    # ---- x^T (host-pretransposed): [p=r_off, i, rb, b], chunked by i,
    #      interleaved with the packed per-slot weights so iteration 0's
    #      matmul chain can start as soon as the first chunks land ----
    xT = singles.tile([128, I, RB, 128], F32)
    w_all = singles.tile([128, I, RB, 48], F32)
    w_re = [w_aps[s].rearrange("(rb p) i o -> p i rb o", p=128)
            for s in range(NSLOT)]
    for i in range(I):
        nc.sync.dma_start(xT[:, i], xT_in[:, i])
        for s in range(NSLOT):
            nc.sync.dma_start(w_all[:, i, :, 16 * s:16 * s + 16], w_re[s][:, i])

    # w_cT_all: [96, 9216]; slot s at partitions 32s..32s+16, (r,i)-flat.
    # Slot 0 first so iteration 1 can start while the rest stream in.
    w_cT = singles.tile([96, K], F32)
    nc.sync.dma_start(w_cT[0:16, :], wT_aps[0])

    # ---- x natural layout [b, r, i] (contiguous per partition) ----
    x_u = singles.tile([128, R, I], F32)
    nc.sync.dma_start(x_u, xh)
    for s in range(1, NSLOT):
        nc.sync.dma_start(w_cT[32 * s:32 * s + 16, :], wT_aps[s])

    # u^T per slot lives at partitions 32s..32s+16 of one [96,128] tile
    uT = singles.tile([96, 128], F32)
    ones_pp = singles.tile([128, 1], F32)
    nc.vector.memset(ones_pp, 1.0)

    u_tiles = [None] * NSLOT

    def squash_tail(s, it, sN_src, d_ap):
        """sN_src [128,16] (psum) + per-partition denom d -> O; update
        u / uT / out."""
        sN = small.tile([128, O], F32, tag="sN")
        nc.vector.tensor_copy(sN, sN_src)

        scr = small.tile([128, O], F32, tag="scr")
        q = small.tile([128, 1], F32, tag="q")
        nc.scalar.square(scr, sN)
        nc.vector.reduce_sum(q, scr, axis=AX.X)
        if d_ap is None:
            rd = ones_pp
        else:
            rd = small.tile([128, 1], F32, tag="rd")
            nc.vector.reciprocal(rd, d_ap)
        rq = small.tile([128, 1], F32, tag="rq")
        nc.scalar.sqrt(rq, q)
        a = small.tile([128, 1], F32, tag="a")
        nc.vector.tensor_mul(a, rq, rd)
        n = small.tile([128, 1], F32, tag="n")
        nc.scalar.square(n, a)
        den = small.tile([128, 1], F32, tag="den")
        nc.scalar.add(den, n, 1.0)
        rden = small.tile([128, 1], F32, tag="rden")
        nc.vector.reciprocal(rden, den)
        t2 = small.tile([128, 1], F32, tag="t2")
        nc.vector.tensor_mul(t2, a, rd)
        gf = small.tile([128, 1], F32, tag="gf")
        nc.vector.tensor_mul(gf, t2, rden)
        Ot = small.tile([128, O], F32, tag=f"O{it}_{s}", bufs=1)
        nc.vector.tensor_scalar_mul(Ot, sN, gf)

        if it == 2:
            nc.sync.dma_start(out_ap[s], Ot)
            return
        if it == 0:
            u_tiles[s] = Ot
        else:
            u2 = small.tile([128, O], F32, tag=f"u2_{s}", bufs=1)
            nc.vector.tensor_add(u2, u_tiles[s], Ot)
            u_tiles[s] = u2
        psu = pst.tile([128, 512], F32, tag="ptr")
        nc.tensor.transpose(psu[0:16, 0:128], u_tiles[s], ident)
        ustg = small.tile([16, 128], F32, tag="ustg")
        nc.scalar.copy(ustg, psu[0:16, 0:128])
        nc.sync.dma_start(uT[32 * s:32 * s + 16, :], ustg)

    # ---- iteration 0: all 3 slots in one packed matmul chain ----
    ps48 = pss.tile([48, 128], F32, tag="pss")
    for idx in range(KT):
        i, rb = idx // RB, idx % RB
        nc.tensor.matmul(
            ps48, lhsT=mmdt(w_all[:, i, rb, :], USE_F32R_S),
            rhs=mmdt(xT[:, i, rb, :], USE_F32R_S),
            start=(idx == 0), stop=(idx == KT - 1),
        )
    sT48 = small.tile([48, 128], F32, tag="sT48")
    nc.scalar.mul(sT48, ps48, 1.0 / R)
    ps2a = pst.tile([128, 512], F32, tag="ptr")
    nc.tensor.transpose(ps2a[:, 0:48], sT48, ident[0:48, 0:48])
    for s in range(NSLOT):
        squash_tail(s, 0, ps2a[:, 16 * s:16 * s + 16], None)

    # ---- iterations 1, 2: software-pipelined across slots.
    # Stage A(s): V-matmuls + x*V + reduce + exp (DVE-heavy).
    # Stage B(s): e^T transposes, y = x^T*e^T, s-matmul, squash (PE/Pool).
    # B(s) instructions are interleaved into A(s+1)'s wave loop so the
    # per-engine static schedule overlaps the stages.
    def stage_B(it, s, e_t, d):
        fast = False
        eT = et_pool.tile([128, RB, 128], F32, tag="eT")
        eTf = eT.rearrange("p rb b -> p (rb b)")
        for g, cnt in ((0, 4), (4, 4), (8, 1)):
            ps = pst.tile([128, 512], F32, tag="ptr")
            for sub in range(cnt):
                rb = g + sub
                nc.tensor.transpose(
                    ps[:, sub * 128:(sub + 1) * 128],
                    e_t[:, rb * 128:(rb + 1) * 128], ident,
                )
            if g % 2 == 0:
                nc.scalar.copy(eTf[:, g * 128:(g + cnt) * 128],
                               ps[:, 0:cnt * 128])
            else:
                nc.vector.tensor_copy(eTf[:, g * 128:(g + cnt) * 128],
                                      ps[:, 0:cnt * 128])
            yield
        ps_s = pss.tile([16, 128], F32, tag="pss")
        for qq in range(4):
            yh = y_pool.tile([128, 2, RB, 128], F32, tag="yh")
            e_bcast = bass.AP(
                tensor=eT.tensor, offset=eT.offset,
                ap=[eT.ap[0], [0, 2], [128, RB], [1, 128]],
            )
            mul_eng = nc.gpsimd if qq % 2 == 0 else nc.vector
            mul_eng.tensor_mul(yh, xT[:, qq * 2:(qq + 1) * 2, :, :], e_bcast)
            for jj in range(18):
                ii, rb = jj // RB, jj % RB
                idx = qq * 18 + jj
                nc.tensor.matmul(
                    ps_s,
                    lhsT=mmdt(w_all[:, qq * 2 + ii, rb, 16 * s:16 * s + 16],
                              fast),
                    rhs=mmdt(yh[:, ii, rb, :], fast),
                    start=(idx == 0), stop=(idx == KT - 1),
                )
            yield
        sT_sb = small.tile([16, 128], F32, tag="sTsb")
        nc.scalar.copy(sT_sb, ps_s)
        ps2 = pst.tile([128, 512], F32, tag="ptr")
        nc.tensor.transpose(ps2[:, 0:16], sT_sb, ident[0:16, 0:16])
        squash_tail(s, it, ps2[:, 0:16], d)
        yield

    def drain(gen, n=None):
        if gen is None:
            return None
        try:
            if n is None:
                while True:
                    next(gen)
            else:
                for _ in range(n):
                    next(gen)
        except StopIteration:
            return None
        return gen

    pending = None
    for it in (1, 2):
        for s in range(NSLOT):
            l_t = le_pool.tile([128, R], F32, tag="l")
            for w9 in range(9):
                pv = psv.tile([128, 1024], F32, tag="pv")
                for cc in range(2):
                    ck = w9 * 2 + cc
                    nc.tensor.matmul(
                        pv[:, cc * 512:(cc + 1) * 512],
                        lhsT=mmdt(uT[32 * s:32 * s + 16, :], USE_F32R_V),
                        rhs=mmdt(w_cT[32 * s:32 * s + 16,
                                      ck * 512:(ck + 1) * 512], USE_F32R_V),
                        start=True, stop=True, tile_position=(32 * s, 0),
                    )
                tw = tw_pool.tile([128, 128, I], F32, tag="tw")
                nc.vector.tensor_mul(
                    tw, x_u[:, w9 * 128:(w9 + 1) * 128, :],
                    pv.rearrange("p (r i) -> p r i", i=I),
                )
                nc.vector.reduce_sum(l_t[:, w9 * 128:(w9 + 1) * 128],
                                     tw, axis=AX.X)
                pending = drain(pending, 1)
            pending = drain(pending)
            e_t = l_t
            d = small.tile([128, 1], F32, tag="d")
            nc.scalar.activation(e_t, l_t, AF.Exp, accum_out=d)
            pending = stage_B(it, s, e_t, d)
    drain(pending)


def build_program():
    from concourse import bacc
    nc = bacc.Bacc("TRN2", target_bir_lowering=False, debug=False,
                   num_devices=NCORES)
    xh = nc.declare_dram_parameter("xh", [BH, R, I], F32, isOutput=False).ap()
    xT_in = nc.declare_dram_parameter("xT", [128, I, RB, BH], F32,
                                      isOutput=False).ap()
    w_aps = [
        nc.declare_dram_parameter(f"w{s}", [R, I, O], F32, isOutput=False).ap()
        for s in range(NSLOT)
    ]
    wT_aps = [
        nc.declare_dram_parameter(f"wT{s}", [O, K], F32, isOutput=False).ap()
        for s in range(NSLOT)
    ]
    out = nc.declare_dram_parameter("out", [NSLOT, BH, O], F32, isOutput=True).ap()
    with tile.TileContext(nc) as tc:
        _caps_kernel(tc, out, xh, xT_in, w_aps, wT_aps)
    nc.compile()
    return nc


def make_in_maps(x, w):
    in_maps = []
    ones_w = np.ones([R, I, O], dtype=np.float32)
    xTs = {}
    for h in range(2):
        xh_np = x[h * BH:(h + 1) * BH]  # [128 b, 1152 r, 8 i]
        # xT[p=r_off, i, rb, b] = xh[b, rb*128+p, i]
        xTs[h] = np.ascontiguousarray(
            xh_np.reshape(BH, RB, 128, I).transpose(2, 3, 1, 0))
    wTs = {c: np.ascontiguousarray(w[c].reshape(K, O).T) for c in range(C)}
    wT_ones = np.ascontiguousarray(ones_w.reshape(K, O).T)
    for k in range(NCORES):
        h, cs = core_assignment(k)
        m = {"xh": np.ascontiguousarray(x[h * BH:(h + 1) * BH]), "xT": xTs[h]}
        for s, c in enumerate(cs):
            m[f"w{s}"] = np.ascontiguousarray(w[c]) if c >= 0 else ones_w
            m[f"wT{s}"] = wTs[c] if c >= 0 else wT_ones
        in_maps.append(m)
    return in_maps


def kernel(x: np.ndarray, route_weights: np.ndarray) -> np.ndarray:
    from concourse.bass_utils import run_bass_kernel_spmd

    x = np.ascontiguousarray(x, dtype=np.float32)
    w = np.ascontiguousarray(route_weights, dtype=np.float32)
    in_maps = make_in_maps(x, w)
    nc = build_program()
    res = run_bass_kernel_spmd(nc, in_maps, list(range(NCORES)))
    global LAST_RESULTS
    LAST_RESULTS = res

    out = np.zeros([C, B, 1, O], dtype=np.float32)
    for k in range(NCORES):
        h, cs = core_assignment(k)
        o = res.results[k]["out"]
        for s, c in enumerate(cs):
            if c >= 0:
                out[c, h * BH:(h + 1) * BH, 0, :] = o[s]
    return out


if __name__ == "__main__":
    rng = np.random.default_rng(0)
    x = rng.normal(size=(B, R, I)).astype(np.float32)
    w = rng.normal(size=(C, R, I, O)).astype(np.float32)
    out = kernel(x=x, route_weights=w)
    print(out.shape, out.dtype, np.abs(out).mean())

